# revision 4
# baseline (speedup 1.0000x reference)
"""Chessboard rearrangement kernel for Trainium2.

Input  [64, 256, 256, 16] f32 -> output [64, 8, 8, 16384] f32 where
out[b, i, j] = inputs[b, i*32:(i+1)*32, j*32:(j+1)*32, :].reshape(-1).

Pure data movement: every output cell gathers 32 strided 2KB chunks from
the input. Implemented as DRAM->DRAM DMA with 3D access patterns; batch
axis sharded 8-way across NeuronCores (8 samples per core).
"""

import sys

sys.path.insert(0, "/opt/trn_rl_repo")

import numpy as np

import concourse.bass as bass
import concourse.mybir as mybir
from concourse.bass_utils import run_bass_kernel_spmd

B, H, W, C = 64, 256, 256, 16
N_CORES = 8
B_PER = B // N_CORES          # 8 samples per core
HC, WC = H // 8, W // 8       # 32, 32 per-cell spatial dims
CELL = HC * WC * C            # 16384 elements per output cell
SAMPLE = H * W * C            # 1048576 elements per sample
ROWBLK = HC * W * C           # 131072 elements per input cell-row block
CHUNK = WC * C                # 512 contiguous elements (2 KB)

_cached = {}


def _build(reps: int = 1):
    if reps in _cached:
        return _cached[reps]
    nc = bass.Bass()
    x = nc.declare_dram_parameter(
        "x", [B_PER, H, W, C], mybir.dt.float32, isOutput=False
    )
    y = nc.declare_dram_parameter(
        "y", [B_PER, 8, 8, CELL], mybir.dt.float32, isOutput=True
    )

    # One DMA per (sample, cell-row): gathers j=8 cells x hc=32 rows of
    # 512-elem chunks into a contiguous 512 KB output block. The output
    # linear offset of block (b, i) equals the input linear offset.
    jobs = [(b * SAMPLE + i * ROWBLK) for b in range(B_PER) for i in range(8)]
    half = len(jobs) // 2

    def emit(eng, offs, sem):
        for r in range(reps):
            for off in offs:
                in_ap = bass.AP(x, off, [[CHUNK, 8], [W * C, HC], [1, CHUNK]])
                out_ap = bass.AP(y, off, [[CELL, 8], [CHUNK, HC], [1, CHUNK]])
                eng.dma_start(out=out_ap, in_=in_ap).then_inc(sem, 16)
        eng.wait_ge(sem, 16 * len(offs) * reps)

    with (
        nc.Block() as block,
        nc.semaphore("sem_sp") as sem_sp,
        nc.semaphore("sem_act") as sem_act,
    ):

        @block.sync
        def _(eng):
            emit(eng, jobs[:half], sem_sp)

        @block.scalar
        def _(eng):
            emit(eng, jobs[half:], sem_act)

    _cached[reps] = nc
    return nc


def kernel(inputs: np.ndarray) -> np.ndarray:
    nc = _build()
    inputs = np.ascontiguousarray(inputs, dtype=np.float32)
    in_maps = [
        {"x": inputs[k * B_PER : (k + 1) * B_PER]} for k in range(N_CORES)
    ]
    res = run_bass_kernel_spmd(nc, in_maps, list(range(N_CORES)))
    out = np.concatenate([res.results[k]["y"] for k in range(N_CORES)], axis=0)
    return out


# revision 5
# speedup vs baseline: 1.1118x; 1.1118x over previous
"""Chessboard rearrangement kernel for Trainium2.

Input  [64, 256, 256, 16] f32 -> output [64, 8, 8, 16384] f32 where
out[b, i, j] = inputs[b, i*32:(i+1)*32, j*32:(j+1)*32, :].reshape(-1).

Pure data movement: every output cell gathers 32 strided 2KB chunks from
the input. Implemented as DRAM->DRAM DMA with 3D access patterns; batch
axis sharded 8-way across NeuronCores (8 samples per core).
"""

import sys

sys.path.insert(0, "/opt/trn_rl_repo")

import numpy as np

import concourse.bass as bass
import concourse.mybir as mybir
from concourse.bass_utils import run_bass_kernel_spmd

B, H, W, C = 64, 256, 256, 16
N_CORES = 8
B_PER = B // N_CORES          # 8 samples per core
HC, WC = H // 8, W // 8       # 32, 32 per-cell spatial dims
CELL = HC * WC * C            # 16384 elements per output cell
SAMPLE = H * W * C            # 1048576 elements per sample
ROWBLK = HC * W * C           # 131072 elements per input cell-row block
CHUNK = WC * C                # 512 contiguous elements (2 KB)

_cached = {}


def _build(reps: int = 1):
    if reps in _cached:
        return _cached[reps]
    nc = bass.Bass()
    x = nc.declare_dram_parameter(
        "x", [B_PER, H, W, C], mybir.dt.float32, isOutput=False
    )
    y = nc.declare_dram_parameter(
        "y", [B_PER, 8, 8, CELL], mybir.dt.float32, isOutput=True
    )

    # One DMA per (sample, cell-row): reads the 512 KB input block
    # linearly and scatter-writes 2 KB chunks into the 8 output cells
    # (iteration order hc, j, chunk). The output linear offset of block
    # (b, i) equals the input linear offset. Linear-read + scattered-write
    # measured ~6% faster than the gather-read + linear-write dual.
    jobs = [(b * SAMPLE + i * ROWBLK) for b in range(B_PER) for i in range(8)]
    half = len(jobs) // 2

    def emit(eng, offs, sem):
        for r in range(reps):
            for off in offs:
                in_ap = bass.AP(x, off, [[1, ROWBLK]])
                out_ap = bass.AP(y, off, [[CHUNK, HC], [CELL, 8], [1, CHUNK]])
                eng.dma_start(out=out_ap, in_=in_ap).then_inc(sem, 16)
        eng.wait_ge(sem, 16 * len(offs) * reps)

    with (
        nc.Block() as block,
        nc.semaphore("sem_sp") as sem_sp,
        nc.semaphore("sem_act") as sem_act,
    ):

        @block.sync
        def _(eng):
            emit(eng, jobs[:half], sem_sp)

        @block.scalar
        def _(eng):
            emit(eng, jobs[half:], sem_act)

    _cached[reps] = nc
    return nc


def kernel(inputs: np.ndarray) -> np.ndarray:
    nc = _build()
    inputs = np.ascontiguousarray(inputs, dtype=np.float32)
    in_maps = [
        {"x": inputs[k * B_PER : (k + 1) * B_PER]} for k in range(N_CORES)
    ]
    res = run_bass_kernel_spmd(nc, in_maps, list(range(N_CORES)))
    out = np.concatenate([res.results[k]["y"] for k in range(N_CORES)], axis=0)
    return out


# revision 6
# speedup vs baseline: 1.1133x; 1.0014x over previous
"""Chessboard rearrangement kernel for Trainium2.

Input  [64, 256, 256, 16] f32 -> output [64, 8, 8, 16384] f32 where
out[b, i, j] = inputs[b, i*32:(i+1)*32, j*32:(j+1)*32, :].reshape(-1).

Pure data movement (memory-bound): the permutation granule is one
2 KB chunk (32 W-pixels x 16 channels). Implemented as direct
DRAM->DRAM DMA with 3D access patterns - per (sample, cell-row) block,
one DMA reads 512 KB linearly and scatter-writes 2 KB chunks into the 8
output cells. Batch axis is sharded 8-way across NeuronCores (8 samples
per core, 32 MiB in + 32 MiB out each, ~187 us HBM roofline at
358 GB/s; measured ~200 us). DMAs are split across both HWDGE queues
(SP + ACT), first half of the samples on SP, second half on ACT.
Measured on HW: linear-read + scattered-write beats the gather-read +
linear-write dual by ~6%, and beats through-SBUF staging (which doubles
SDMA work) by ~15%.
"""

import sys

sys.path.insert(0, "/opt/trn_rl_repo")

import numpy as np

import concourse.bass as bass
import concourse.mybir as mybir
from concourse.bass_utils import run_bass_kernel_spmd

B, H, W, C = 64, 256, 256, 16
N_CORES = 8
B_PER = B // N_CORES          # 8 samples per core
HC, WC = H // 8, W // 8       # 32, 32 per-cell spatial dims
CELL = HC * WC * C            # 16384 elements per output cell
SAMPLE = H * W * C            # 1048576 elements per sample
ROWBLK = HC * W * C           # 131072 elements per input cell-row block
CHUNK = WC * C                # 512 contiguous elements (2 KB)

_cached = {}


def _build(reps: int = 1):
    if reps in _cached:
        return _cached[reps]
    nc = bass.Bass()
    x = nc.declare_dram_parameter(
        "x", [B_PER, H, W, C], mybir.dt.float32, isOutput=False
    )
    y = nc.declare_dram_parameter(
        "y", [B_PER, 8, 8, CELL], mybir.dt.float32, isOutput=True
    )

    # One DMA per (sample, cell-row): reads the 512 KB input block
    # linearly and scatter-writes 2 KB chunks into the 8 output cells
    # (iteration order hc, j, chunk). The output linear offset of block
    # (b, i) equals the input linear offset. Linear-read + scattered-write
    # measured ~6% faster than the gather-read + linear-write dual.
    jobs = [(b * SAMPLE + i * ROWBLK) for b in range(B_PER) for i in range(8)]
    half = len(jobs) // 2

    def emit(eng, offs, sem):
        for r in range(reps):
            for off in offs:
                in_ap = bass.AP(x, off, [[1, ROWBLK]])
                out_ap = bass.AP(y, off, [[CHUNK, HC], [CELL, 8], [1, CHUNK]])
                eng.dma_start(out=out_ap, in_=in_ap).then_inc(sem, 16)
        eng.wait_ge(sem, 16 * len(offs) * reps)

    with (
        nc.Block() as block,
        nc.semaphore("sem_sp") as sem_sp,
        nc.semaphore("sem_act") as sem_act,
    ):

        @block.sync
        def _(eng):
            emit(eng, jobs[:half], sem_sp)

        @block.scalar
        def _(eng):
            emit(eng, jobs[half:], sem_act)

    _cached[reps] = nc
    return nc


def kernel(inputs: np.ndarray) -> np.ndarray:
    nc = _build()
    inputs = np.ascontiguousarray(inputs, dtype=np.float32)
    in_maps = [
        {"x": inputs[k * B_PER : (k + 1) * B_PER]} for k in range(N_CORES)
    ]
    res = run_bass_kernel_spmd(nc, in_maps, list(range(N_CORES)))
    out = np.concatenate([res.results[k]["y"] for k in range(N_CORES)], axis=0)
    return out
